# revision 106
# baseline (speedup 1.0000x reference)
"""GQA causal attention (llama3-style RoPE) on 8 TRN2 NeuronCores.

Sharding: tensor-parallel over heads. Core c gets q-heads 4c..4c+3 and
kv-head c (GQA groups intact), plus the matching row-block of wo.T.
Each core computes a full [S, D] partial of the output projection; the
host sums the 8 partials (the "all-reduce" of the row-sharded wo).

Per-core pipeline:
  qkvT = wqkvT.T @ xT        fp8e4 DoubleRow (K=256/tile) with hi+lo
                             error compensation: x*w ~= xh*wh + xl*wh
                             + xh*wl -> ~bf16 accuracy at 0.75x the PE
                             cycles of a bf16 matmul chain.
  RoPE on qT/kT (de-interleaved [R|I] head layout via host-permuted
                             weight cols; 1/32 weight scale folded into
                             the trig tables), final add writes fp8
                             directly into DoubleRow layout [64,2,S]
                             (dead slot 1 zeroed once; K padded 64->128)
  sT[sk, sq] = kT.T @ qT     fp8e4 DoubleRow: 0.5 cyc/col
  eT = exp(sT/8) * mask      ACT, psum->bf16 SBUF
  avT[sq, 65] = eT.T @ vaug  et stationary, v_aug moving: 65 cols/tile
                             instead of 512 (cost model charges moving
                             cols only). vaug col 64 = 32.0 (folds the
                             1/32 v descale into the softmax denom).
  y = avT[:, 0:64] * recip   DVE recip + Pool per-partition scale
  yT via DMA-transpose       (SBUF xbar, no PE/psum cost)
  out[sq, d] = yT.T @ woT    bf16; drains DVE/Pool -> f16; host sums.
"""

import sys

for _p in ("/opt/trn_rl_repo", "/root/.axon_site/_ro/trn_rl_repo"):
    if _p not in sys.path:
        sys.path.insert(0, _p)

import numpy as np
import ml_dtypes

import concourse.bass as bass
import concourse.bacc as bacc
import concourse.mybir as mybir
import concourse.tile as tile

BF16 = ml_dtypes.bfloat16
F8 = ml_dtypes.float8_e4m3

S = 2048
D = 2048
HD = 64
NH = 32
NKV = 8
NCORES = 8
QH = NH // NCORES            # 4 local q heads
QCOLS = QH * HD              # 256
KVCOLS = 2 * HD              # 128 (k and v, one kv head)
COLS = QCOLS + KVCOLS        # 384
P = 128                      # partitions
NKT = 8                      # DoubleRow contraction tiles (K=256 each)
NSQ = S // P                 # 16 seq tiles of 128
NCH = 4                      # seq chunks of 512
CH = 512
WSCALE = 32.0                # fp8 range scale on wq/wk/wv

_CACHE = {}


def _build():
    f8 = mybir.dt.float8e4
    bf = mybir.dt.bfloat16
    f16 = mybir.dt.float16
    f32 = mybir.dt.float32
    DR = mybir.MatmulPerfMode.DoubleRow

    nc = bacc.Bacc()
    x8h_d = nc.dram_tensor("x8h", [P, NKT, 2, S], f8, kind="ExternalInput")
    x8l_d = nc.dram_tensor("x8l", [P, NKT, 2, S], f8, kind="ExternalInput")
    w8h_d = nc.dram_tensor("w8h", [P, NKT, 2, COLS], f8, kind="ExternalInput")
    w8l_d = nc.dram_tensor("w8l", [P, NKT, 2, COLS], f8, kind="ExternalInput")
    wot8h_d = nc.dram_tensor("wot8h", [P, 2, D], f8, kind="ExternalInput")
    wot8l_d = nc.dram_tensor("wot8l", [P, 2, D], f8, kind="ExternalInput")
    cos_d = nc.dram_tensor("cos128", [P, S], f16, kind="ExternalInput")
    swap_d = nc.dram_tensor("swap128", [P, S], f16, kind="ExternalInput")
    masks_d = nc.dram_tensor("masks", [P, P], bf, kind="ExternalInput")
    zeros_d = nc.dram_tensor("zeros8", [HD, S], f8, kind="ExternalInput")
    id64_d = nc.dram_tensor("ident64", [HD, HD], f16, kind="ExternalInput")
    id128_d = nc.dram_tensor("ident128", [P, P], bf, kind="ExternalInput")
    out_d = nc.dram_tensor("out", [S, D], f16, kind="ExternalOutput")

    with tile.TileContext(nc) as tc:
        with (
            tc.tile_pool(name="const", bufs=1) as cpool,
            tc.tile_pool(name="x8", bufs=1) as xpool,
            tc.tile_pool(name="w8", bufs=1) as wpool,
            tc.tile_pool(name="big", bufs=1) as bigpool,
            tc.tile_pool(name="vaug", bufs=NSQ) as vpool,
            tc.tile_pool(name="et", bufs=18) as epool,
            tc.tile_pool(name="tmp", bufs=3) as tpool,
            tc.tile_pool(name="ysb", bufs=4) as ypool,
            tc.tile_pool(name="ot", bufs=2) as opool,
            tc.tile_pool(name="ps_a", bufs=2, space="PSUM") as ps_a,
            tc.tile_pool(name="ps_s", bufs=2, space="PSUM") as ps_s,
            tc.tile_pool(name="ps_av", bufs=2, space="PSUM") as ps_av,
        ):
            # proj-critical loads first on the serial DMA resource: weights,
            # then x in chunk-quarters so proj chunk j starts when quarter
            # j lands; small tables (needed later) afterwards via SWDGE
            w8h_sb = wpool.tile([P, NKT, 2, COLS], f8, tag="w8h")
            w8l_sb = wpool.tile([P, NKT, 2, COLS], f8, tag="w8l")
            x8h_sb = xpool.tile([P, NKT, 2, S], f8, tag="x8h")
            x8l_sb = xpool.tile([P, NKT, 2, S], f8, tag="x8l")
            # emission order = serial DMA-resource order; front-load exactly
            # what the first (kv, chunk 0) projection pass needs
            nc.sync.dma_start(w8h_sb[:, :, :, 2 * P :], w8h_d[:, :, :, 2 * P :])
            nc.sync.dma_start(x8h_sb[:, :, :, 0:CH], x8h_d[:, :, :, 0:CH])
            nc.scalar.dma_start(x8l_sb[:, :, :, 0:CH], x8l_d[:, :, :, 0:CH])
            nc.scalar.dma_start(w8l_sb[:, :, :, 2 * P :], w8l_d[:, :, :, 2 * P :])
            nc.sync.dma_start(w8h_sb[:, :, :, : 2 * P], w8h_d[:, :, :, : 2 * P])
            nc.scalar.dma_start(w8l_sb[:, :, :, : 2 * P], w8l_d[:, :, :, : 2 * P])
            for q in range(1, NCH):
                qs = slice(q * CH, (q + 1) * CH)
                nc.sync.dma_start(x8h_sb[:, :, :, qs], x8h_d[:, :, :, qs])
                nc.scalar.dma_start(x8l_sb[:, :, :, qs], x8l_d[:, :, :, qs])

            cos_sb = cpool.tile([P, S], f16, tag="cos")
            swap_sb = cpool.tile([P, S], f16, tag="swap")
            masks_sb = cpool.tile([P, P], bf, tag="masks")
            id64_sb = cpool.tile([HD, HD], f16, tag="id64")
            id128_sb = cpool.tile([P, P], bf, tag="id128")
            zbias = cpool.tile([P, 1], f32, tag="zbias")
            nc.gpsimd.memset(zbias[:], 0.0)
            nc.gpsimd.dma_start(cos_sb[:], cos_d[:])
            nc.gpsimd.dma_start(swap_sb[:], swap_d[:])
            nc.gpsimd.dma_start(masks_sb[:], masks_d[:])
            nc.gpsimd.dma_start(id64_sb[:], id64_d[:])
            nc.gpsimd.dma_start(id128_sb[:], id128_d[:])

            wot8h_sb = cpool.tile([P, 2, D], f8, tag="wot8h")
            wot8l_sb = cpool.tile([P, 2, D], f8, tag="wot8l")
            nc.gpsimd.dma_start(wot8h_sb[:], wot8h_d[:])
            nc.gpsimd.dma_start(wot8l_sb[:], wot8l_d[:])

            # q/k fp8 DoubleRow tiles: slot 0 written by rope, slot 1 dead
            # but must be exact zeros (it pairs with zeroed lhsT rows; and
            # junk fp8 bytes can be NaN)
            qt8 = [bigpool.tile([HD, 2, S], f8, tag=f"qt{h}", name=f"qt{h}") for h in range(QH)]
            kt8 = bigpool.tile([HD, 2, S], f8, tag="kt")
            for h in range(QH):
                nc.gpsimd.dma_start(qt8[h][:, 1, :], zeros_d[:])
            nc.gpsimd.dma_start(kt8[:, 1, :], zeros_d[:])

            vt_sb = bigpool.tile([HD, S], f16, tag="vt")
            yt_sb = [bigpool.tile([P, S], bf, tag=f"yt{m}", name=f"yt{m}") for m in range(2)]
            y8h_sb = bigpool.tile([P, 2, S], f8, tag="y8h")
            y8l_sb = bigpool.tile([P, 2, S], f8, tag="y8l")

            vaug_sb = [None] * NSQ

            # ---- projection chunk: 8 DoubleRow k-tiles x (xh*wh + xl*wh
            # + xh*wl), then RoPE straight into the fp8 DR tiles
            def proj_pass(ps, m, j, xi):
                chunk = slice(j * CH, (j + 1) * CH)
                mc = slice(m * P, (m + 1) * P)
                xs, ws = ((x8h_sb, w8h_sb), (x8l_sb, w8h_sb), (x8h_sb, w8l_sb))[xi]
                with nc.named_scope("proj"):
                    for kt in range(NKT):
                        nc.tensor.matmul(
                            ps[:],
                            ws[:, kt, :, mc],
                            xs[:, kt, :, chunk],
                            start=(kt == 0 and xi == 0),
                            stop=(kt == NKT - 1 and xi == 2),
                            perf_mode=DR,
                        )

            def proj_chunk(m, j):
                chunk = slice(j * CH, (j + 1) * CH)
                ps = ps_a.tile([P, CH], f32, tag="proj", name="ps_proj")
                # hi*hi pass first so the chain can start before the
                # lo-tensors finish loading
                for xi in range(3):
                    proj_pass(ps, m, j, xi)
                qr = tpool.tile([P, CH], f16, tag="rope_qr", name="rope_qr")
                with nc.named_scope("rope"):
                    nc.vector.tensor_copy(qr[:], ps[:])
                    t2 = tpool.tile([P, CH], f16, tag="rope_t2", name="rope_t2")
                    t3 = tpool.tile([P, CH], f16, tag="rope_t3", name="rope_t3")
                    if m < 2:
                        # rows: [R(h0) I(h0) R(h1) I(h1)] blocks of 32;
                        # out block R <- I-in * (-sin), I <- R-in * (+sin)
                        for b in range(4):
                            s0 = b * 32 + (32 if b % 2 == 0 else -32)
                            nc.gpsimd.tensor_mul(
                                t2[b * 32 : b * 32 + 32, :],
                                qr[s0 : s0 + 32, :],
                                swap_sb[s0 : s0 + 32, chunk],
                            )
                        nc.vector.tensor_mul(t3[:], qr[:], cos_sb[:, chunk])
                        for hh in range(2):
                            nc.gpsimd.tensor_add(
                                qt8[2 * m + hh][:, 0, chunk],
                                t3[hh * HD : (hh + 1) * HD, :],
                                t2[hh * HD : (hh + 1) * HD, :],
                            )
                    else:
                        # rows 0-63: [R(k) I(k)]; rows 64-127: v (natural)
                        nc.gpsimd.tensor_mul(t2[0:32, :], qr[32:64, :], swap_sb[32:64, chunk])
                        nc.gpsimd.tensor_mul(t2[32:64, :], qr[0:32, :], swap_sb[0:32, chunk])
                        nc.vector.tensor_mul(t3[0:HD, :], qr[0:HD, :], cos_sb[0:HD, chunk])
                        nc.gpsimd.tensor_add(kt8[:, 0, chunk], t3[0:HD, :], t2[0:HD, :])
                        nc.gpsimd.tensor_copy(vt_sb[:, chunk], qr[HD:P, :])

            ysb_tiles = {}

            # ---- scores + exp for one (chunk, head), weaving queued work
            # items (av chains / wo row-blocks) between score pairs so the
            # PE has filler while ACT chews through the exps
            def sdpa_scores(j, h, work):
                nlive = 4 * j + 4
                offs = [max(0, (i - 4 * j)) * P for i in range(nlive)]
                ets = []
                for i in range(0, nlive, 2):
                    with nc.named_scope("scores"):
                        ps2 = ps_s.tile([P, 2 * CH], f32, tag="sc", name="ps_sc")
                        o0, o1 = offs[i], offs[i + 1]
                        # u=1 output is shifted left by o1 so the pair's
                        # live region [o0, 2CH-o1) is contiguous and a
                        # single exp op covers it (av adjusts its indexing
                        # via the same shift)
                        nc.tensor.matmul(
                            ps2[:, o0:CH],
                            kt8[:, :, i * P : (i + 1) * P],
                            qt8[h][:, :, j * CH + o0 : (j + 1) * CH],
                            start=True,
                            stop=True,
                            perf_mode=DR,
                        )
                        nc.tensor.matmul(
                            ps2[:, CH : 2 * CH - o1],
                            kt8[:, :, (i + 1) * P : (i + 2) * P],
                            qt8[h][:, :, j * CH + o1 : (j + 1) * CH],
                            start=True,
                            stop=True,
                            perf_mode=DR,
                        )
                        et2 = epool.tile([P, 2 * CH], bf, tag="et", name="et")
                        with nc.named_scope("exp"):
                            nc.scalar.activation(
                                et2[:, o0 : 2 * CH - o1],
                                ps2[:, o0 : 2 * CH - o1],
                                mybir.ActivationFunctionType.Exp,
                                bias=zbias[:],
                                scale=0.125,
                            )
                        for u in range(2):
                            if i + u >= nlive - 4:  # diagonal tile: tri mask
                                off = u * CH + offs[i + u] - (o1 if u else 0)
                                with nc.named_scope("mask"):
                                    nc.gpsimd.tensor_mul(
                                        et2[:, off : off + P],
                                        et2[:, off : off + P],
                                        masks_sb[:],
                                    )
                        ets.append(et2)
                    pairs_left = (nlive - i) // 2 - 1
                    pops = 1
                    if pairs_left > 0:
                        pops = max(1, -(-len(work) // (pairs_left + 1)) - 1)
                    for _ in range(min(pops, len(work))):
                        work.pop(0)()
                return ets

            # ---- one flipped-AV chain + normalize for sq-tile t; the last
            # head also fires the tile's y transpose
            def av_tile(j, h, ets, t):
                tl = (t % 4) * P
                pav = ps_av.tile([P, HD + 1], f32, tag="av", name="ps_av")
                with nc.named_scope("av"):
                    for i in range(t + 1):
                        # odd tiles of a pair are stored shifted left by
                        # their causal offset (see sdpa_scores)
                        c0 = (i % 2) * CH + tl - ((i % 2) * max(0, i - 4 * j) * P)
                        nc.tensor.matmul(
                            pav[:],
                            ets[i // 2][:, c0 : c0 + P],
                            vaug_sb[i][:],
                            start=(i == 0),
                            stop=(i == t),
                        )
                with nc.named_scope("norm"):
                    recip = tpool.tile([P, 1], f32, tag="recip", name="recip")
                    nc.vector.reciprocal(recip[:], pav[:, HD : HD + 1])
                    ysb = ysb_tiles[t][h // 2]
                    nc.vector.tensor_scalar_mul(
                        ysb[:, (h % 2) * HD : (h % 2) * HD + HD],
                        pav[:, 0:HD],
                        recip[:],
                    )
                if h == QH - 1:
                    ytrans_tile(t, fast=(j == NCH - 1))

            def ytrans_tile(t, fast=False):
                # SBUF->SBUF DMA transpose (xbar), then Pool splits yT into
                # an fp8 hi+lo pair in DoubleRow layout for the fp8 wo.
                # fast=True (final chunk) uses a PE transpose + DVE drain
                # instead: ~0.5us latency vs ~2.4us for the DMA path.
                ts_ = slice(t * P, (t + 1) * P)
                with nc.named_scope("ytrans"):
                    for k in range(2):
                        if fast:
                            pt = ps_av.tile([P, P], bf, tag="av", name="ps_yt")
                            nc.tensor.transpose(pt[:], ysb_tiles[t][k][:], id128_sb[:])
                            nc.vector.tensor_copy(yt_sb[k][:, ts_], pt[:])
                        else:
                            nc.sync.dma_start(
                                yt_sb[k][:, ts_], ysb_tiles[t][k][:], transpose=True
                            )
                        nc.gpsimd.tensor_copy(y8h_sb[:, k, ts_], yt_sb[k][:, ts_])
                        nc.vector.scalar_tensor_tensor(
                            y8l_sb[:, k, ts_],
                            yt_sb[k][:, ts_],
                            1.0,
                            y8h_sb[:, k, ts_],
                            mybir.AluOpType.mult,
                            mybir.AluOpType.subtract,
                        )

            ot_live = {}

            def wo_dch(sm, dcJ):
                # one [128, 512] block of the output projection: fp8
                # DoubleRow hi+lo (y*w ~= yh*wh + yl*wh + yh*wl); the drain
                # folds in the 1/32 fp8 weight scale
                srow = slice(sm * P, (sm + 1) * P)
                if dcJ == 0:
                    ot_live[sm] = opool.tile([P, D], f16, tag="ot", name="ot")
                ot = ot_live[sm]
                dch = slice(dcJ * CH, (dcJ + 1) * CH)
                pw = ps_a.tile([P, CH], f32, tag="proj", name="ps_wo")
                with nc.named_scope("wo"):
                    for wi, (ys, ws) in enumerate(
                        ((y8h_sb, wot8h_sb), (y8l_sb, wot8h_sb), (y8h_sb, wot8l_sb))
                    ):
                        nc.tensor.matmul(
                            pw[:],
                            ys[:, :, srow],
                            ws[:, :, dch],
                            start=(wi == 0),
                            stop=(wi == 2),
                            perf_mode=DR,
                        )
                with nc.named_scope("outdrain"):
                    # in the final chunk ACT is idle; alternating drains
                    # doubles the drain rate that throttles the tail
                    if sm >= 4 * (NCH - 1) and dcJ % 2 == 0:
                        nc.scalar.activation(
                            ot[:, dch],
                            pw[:],
                            mybir.ActivationFunctionType.Copy,
                            scale=1.0 / WSCALE,
                        )
                    else:
                        nc.vector.tensor_scalar_mul(ot[:, dch], pw[:], 1.0 / WSCALE)
                half = D // 2
                if sm >= 4 * (NCH - 1) and dcJ == 1:
                    # final chunk: ship the first half as soon as it drains
                    with nc.named_scope("outdma"):
                        nc.sync.dma_start(out_d[srow, 0:half], ot[:, 0:half])
                elif dcJ == NCH - 1:
                    with nc.named_scope("outdma"):
                        if sm >= 4 * (NCH - 1):
                            nc.sync.dma_start(out_d[srow, half:D], ot[:, half:D])
                        else:
                            nc.sync.dma_start(out_d[srow, :], ot[:])
                        del ot_live[sm]

            def wo_tile(sm):
                for dcJ in range(NCH):
                    wo_dch(sm, dcJ)

            def vtrans_chunk(j):
                with nc.named_scope("vtrans"):
                    for i in range(4 * j, 4 * j + 4):
                        va = vpool.tile([P, HD + 1], f16, tag="vaug", name=f"vaug{i}")
                        nc.sync.dma_start(
                            va[:, 0:HD], vt_sb[:, i * P : (i + 1) * P], transpose=True
                        )
                        nc.gpsimd.memset(va[:, HD : HD + 1], WSCALE)
                        vaug_sb[i] = va

            # ---- emission: ascending chunks. Everything except scores
            # (projections for the NEXT chunk, av chains of the previous
            # head, wo row-blocks) rides in a FIFO of work items that
            # sdpa_scores pops between score pairs, so the PE always has
            # filler while ACT chews exps; any leftovers are force-drained
            # right after a chunk's last scores (while its exps still run).
            proj_chunk(2, 0)
            vtrans_chunk(0)
            proj_chunk(0, 0)
            proj_chunk(1, 0)
            prev = None  # (j, h, ets)
            work = []
            urgent = []
            pending_wo = []
            seq = [0, 1, 2, 3]
            # per slot (j, h): projection work for the next chunk
            projq = {}
            for jj in range(NCH - 1):
                projq[(jj, 0)] = [
                    lambda jj=jj: proj_chunk(2, jj + 1),
                    lambda jj=jj: vtrans_chunk(jj + 1),
                ]
                projq[(jj, 1)] = [lambda jj=jj: proj_chunk(0, jj + 1)]
                projq[(jj, 2)] = [lambda jj=jj: proj_chunk(1, jj + 1)]
            for j in seq:
                for t in range(4 * j, 4 * j + 4):
                    ysb_tiles[t] = [
                        ypool.tile([P, P], bf, tag=f"ysb{k}", name=f"ysb{k}_{t}")
                        for k in range(2)
                    ]
                for h in range(QH):
                    work.extend(pending_wo)
                    pending_wo = []
                    if prev is not None:
                        pj, ph, pets = prev
                        for t in range(4 * pj, 4 * pj + 4):
                            work.append(
                                lambda pj=pj, ph=ph, pets=pets, t=t: av_tile(
                                    pj, ph, pets, t
                                )
                            )
                        if ph == QH - 1:
                            # one-slot delay so the ytrans+fp8-cast chain
                            # completes before the wo matmuls want it
                            pending_wo = [
                                (lambda t=t, d=d: wo_dch(t, d))
                                for t in range(4 * pj, 4 * pj + 4)
                                for d in range(NCH)
                            ]
                    for it in reversed(projq.get((j, h), [])):
                        work.insert(0, it)
                    ets = sdpa_scores(j, h, work)
                    prev = (j, h, ets)
                # force any not-yet-popped next-chunk projections (and
                # stragglers) out now, under this chunk's trailing exps
                while work:
                    work.pop(0)()
            # final head: per-tile av/norm/ytrans then wo
            while work:
                work.pop(0)()
            for w in pending_wo:
                w()
            pending_wo = []
            fj, fh, fets = prev
            for t in range(4 * fj, 4 * fj + 4):
                av_tile(fj, fh, fets, t)
                if t > 4 * fj:
                    wo_tile(t - 1)
            wo_tile(4 * fj + 3)

    nc.finalize()
    return nc


def _host_inputs(x, freqs_cos, freqs_sin, wq, wk, wv, wo):
    """Build the 8 per-core input maps (all host-side preprocessing)."""
    x = np.asarray(x, np.float32)
    cos = np.asarray(freqs_cos, np.float32)  # [S, 32]
    sin = np.asarray(freqs_sin, np.float32)
    wq = np.asarray(wq, np.float32)
    wk = np.asarray(wk, np.float32)
    wv = np.asarray(wv, np.float32)
    wo = np.asarray(wo, np.float32)

    # de-interleave pairs into [R | I] blocks of 32 within each head
    perm = np.concatenate([np.arange(0, HD, 2), np.arange(1, HD, 2)])

    xt = np.ascontiguousarray(x[0].T)  # [D, S]

    def dr8(a):
        """[D, cols] -> hi/lo fp8 pair in DoubleRow layout [P, NKT, 2, cols]."""
        hi = a.astype(F8)
        lo = (a - hi.astype(np.float32)).astype(F8)
        out = []
        for t in (hi, lo):
            out.append(
                np.ascontiguousarray(
                    t.reshape(NKT, 2, P, -1).transpose(2, 0, 1, 3)
                )
            )
        return out

    x8h, x8l = dr8(xt)

    # trig tables with the 1/32 fp8 weight scale folded in;
    # rows (mod 64): [ +sin (R rows, feeds I-out) is NOT how the mul reads:
    # table row follows the INPUT partition: rows 0-31 (R in) -> +sin,
    # rows 32-63 (I in) -> -sin ]; cos everywhere.
    cos128 = np.empty((P, S), np.float16)
    swap128 = np.empty((P, S), np.float16)
    for dd in range(P):
        i = dd % 32
        cos128[dd] = cos[:, i] / WSCALE
        swap128[dd] = (sin[:, i] if (dd % HD) < 32 else -sin[:, i]) / WSCALE

    pp = np.arange(P)[:, None]
    ff = np.arange(P)[None, :]
    masks = (pp <= ff).astype(np.float32).astype(BF16)
    zeros8 = np.zeros((HD, S), F8)
    ident64 = np.eye(HD, dtype=np.float16)
    ident128 = np.eye(P, dtype=np.float32).astype(BF16)

    in_maps = []
    for c in range(NCORES):
        wq_c = wq[c * QCOLS : (c + 1) * QCOLS].reshape(QH, HD, D)[:, perm, :].reshape(
            QCOLS, D
        )
        wk_c = wk[c * HD : (c + 1) * HD][perm, :]
        wv_c = wv[c * HD : (c + 1) * HD]
        wqkvt = np.ascontiguousarray(
            np.concatenate([wq_c, wk_c, wv_c], axis=0).T
        ) * WSCALE  # [D, COLS]
        w8h, w8l = dr8(wqkvt)
        # wo.T in DoubleRow layout [P, 2, D] (slot = head-pair), x32 scale
        wot = np.ascontiguousarray(
            wo[:, c * QCOLS : (c + 1) * QCOLS].T.reshape(2, P, D).transpose(1, 0, 2)
        ) * WSCALE
        wot8h = wot.astype(F8)
        wot8l = (wot - wot8h.astype(np.float32)).astype(F8)
        in_maps.append(
            {
                "x8h": x8h,
                "x8l": x8l,
                "w8h": w8h,
                "w8l": w8l,
                "wot8h": wot8h,
                "wot8l": wot8l,
                "cos128": cos128,
                "swap128": swap128,
                "masks": masks,
                "zeros8": zeros8,
                "ident64": ident64,
                "ident128": ident128,
            }
        )
    return in_maps


def kernel(x, freqs_cos, freqs_sin, wq, wk, wv, wo):
    from concourse.bass_utils import run_bass_kernel_spmd

    if "nc" not in _CACHE:
        _CACHE["nc"] = _build()
    nc = _CACHE["nc"]
    in_maps = _host_inputs(x, freqs_cos, freqs_sin, wq, wk, wv, wo)
    res = run_bass_kernel_spmd(nc, in_maps, core_ids=list(range(NCORES)))
    out = np.zeros((S, D), np.float64)
    for r in res.results:
        out += r["out"].astype(np.float64)
    return out.astype(np.float32).reshape(1, S, D)


# revision 109
# speedup vs baseline: 1.0025x; 1.0025x over previous
"""GQA causal attention (llama3-style RoPE) on 8 TRN2 NeuronCores.

Sharding: tensor-parallel over heads. Core c gets q-heads 4c..4c+3 and
kv-head c (GQA groups intact), plus the matching row-block of wo.T.
Each core computes a full [S, D] partial of the output projection; the
host sums the 8 partials (the "all-reduce" of the row-sharded wo).

Per-core pipeline:
  qkvT = wqkvT.T @ xT        fp8e4 DoubleRow (K=256/tile) with hi+lo
                             error compensation: x*w ~= xh*wh + xl*wh
                             + xh*wl -> ~bf16 accuracy at 0.75x the PE
                             cycles of a bf16 matmul chain.
  RoPE on qT/kT (de-interleaved [R|I] head layout via host-permuted
                             weight cols; 1/32 weight scale folded into
                             the trig tables), final add writes fp8
                             directly into DoubleRow layout [64,2,S]
                             (dead slot 1 zeroed once; K padded 64->128)
  sT[sk, sq] = kT.T @ qT     fp8e4 DoubleRow: 0.5 cyc/col
  eT = exp(sT/8) * mask      ACT, psum->bf16 SBUF
  avT[sq, 65] = eT.T @ vaug  et stationary, v_aug moving: 65 cols/tile
                             instead of 512 (cost model charges moving
                             cols only). vaug col 64 = 32.0 (folds the
                             1/32 v descale into the softmax denom).
  y = avT[:, 0:64] * recip   DVE recip + Pool per-partition scale
  yT via DMA-transpose       (SBUF xbar, no PE/psum cost)
  out[sq, d] = yT.T @ woT    bf16; drains DVE/Pool -> f16; host sums.
"""

import sys

for _p in ("/opt/trn_rl_repo", "/root/.axon_site/_ro/trn_rl_repo"):
    if _p not in sys.path:
        sys.path.insert(0, _p)

import numpy as np
import ml_dtypes

import concourse.bass as bass
import concourse.bacc as bacc
import concourse.mybir as mybir
import concourse.tile as tile

BF16 = ml_dtypes.bfloat16
F8 = ml_dtypes.float8_e4m3

S = 2048
D = 2048
HD = 64
NH = 32
NKV = 8
NCORES = 8
QH = NH // NCORES            # 4 local q heads
QCOLS = QH * HD              # 256
KVCOLS = 2 * HD              # 128 (k and v, one kv head)
COLS = QCOLS + KVCOLS        # 384
P = 128                      # partitions
NKT = 8                      # DoubleRow contraction tiles (K=256 each)
NSQ = S // P                 # 16 seq tiles of 128
NCH = 4                      # seq chunks of 512
CH = 512
WSCALE = 32.0                # fp8 range scale on wq/wk/wv

_CACHE = {}


def _build():
    f8 = mybir.dt.float8e4
    bf = mybir.dt.bfloat16
    f16 = mybir.dt.float16
    f32 = mybir.dt.float32
    DR = mybir.MatmulPerfMode.DoubleRow

    nc = bacc.Bacc()
    x8h_d = nc.dram_tensor("x8h", [P, NKT, 2, S], f8, kind="ExternalInput")
    x8l_d = nc.dram_tensor("x8l", [P, NKT, 2, S], f8, kind="ExternalInput")
    w8h_d = nc.dram_tensor("w8h", [P, NKT, 2, COLS], f8, kind="ExternalInput")
    w8l_d = nc.dram_tensor("w8l", [P, NKT, 2, COLS], f8, kind="ExternalInput")
    wot8h_d = nc.dram_tensor("wot8h", [P, 2, D], f8, kind="ExternalInput")
    wot8l_d = nc.dram_tensor("wot8l", [P, 2, D], f8, kind="ExternalInput")
    cos_d = nc.dram_tensor("cos128", [P, S], f16, kind="ExternalInput")
    swap_d = nc.dram_tensor("swap128", [P, S], f16, kind="ExternalInput")
    masks_d = nc.dram_tensor("masks", [P, P], bf, kind="ExternalInput")
    zeros_d = nc.dram_tensor("zeros8", [HD, S], f8, kind="ExternalInput")
    id64_d = nc.dram_tensor("ident64", [HD, HD], f16, kind="ExternalInput")
    id128_d = nc.dram_tensor("ident128", [P, P], bf, kind="ExternalInput")
    out_d = nc.dram_tensor("out", [S, D], f16, kind="ExternalOutput")

    with tile.TileContext(nc) as tc:
        with (
            tc.tile_pool(name="const", bufs=1) as cpool,
            tc.tile_pool(name="x8", bufs=1) as xpool,
            tc.tile_pool(name="w8", bufs=1) as wpool,
            tc.tile_pool(name="big", bufs=1) as bigpool,
            tc.tile_pool(name="vaug", bufs=NSQ) as vpool,
            tc.tile_pool(name="et", bufs=18) as epool,
            tc.tile_pool(name="tmp", bufs=3) as tpool,
            tc.tile_pool(name="ysb", bufs=4) as ypool,
            tc.tile_pool(name="ot", bufs=2) as opool,
            tc.tile_pool(name="ps_a", bufs=2, space="PSUM") as ps_a,
            tc.tile_pool(name="ps_s", bufs=2, space="PSUM") as ps_s,
            tc.tile_pool(name="ps_av", bufs=2, space="PSUM") as ps_av,
        ):
            # proj-critical loads first on the serial DMA resource: weights,
            # then x in chunk-quarters so proj chunk j starts when quarter
            # j lands; small tables (needed later) afterwards via SWDGE
            w8h_sb = wpool.tile([P, NKT, 2, COLS], f8, tag="w8h")
            w8l_sb = wpool.tile([P, NKT, 2, COLS], f8, tag="w8l")
            x8h_sb = xpool.tile([P, NKT, 2, S], f8, tag="x8h")
            x8l_sb = xpool.tile([P, NKT, 2, S], f8, tag="x8l")
            # emission order = serial DMA-resource order; front-load exactly
            # what the first (kv, chunk 0) projection pass needs
            nc.sync.dma_start(w8h_sb[:, :, :, 2 * P :], w8h_d[:, :, :, 2 * P :])
            nc.sync.dma_start(x8h_sb[:, :, :, 0:CH], x8h_d[:, :, :, 0:CH])
            nc.scalar.dma_start(x8l_sb[:, :, :, 0:CH], x8l_d[:, :, :, 0:CH])
            nc.scalar.dma_start(w8l_sb[:, :, :, 2 * P :], w8l_d[:, :, :, 2 * P :])
            nc.sync.dma_start(w8h_sb[:, :, :, : 2 * P], w8h_d[:, :, :, : 2 * P])
            nc.scalar.dma_start(w8l_sb[:, :, :, : 2 * P], w8l_d[:, :, :, : 2 * P])
            for q in range(1, NCH):
                qs = slice(q * CH, (q + 1) * CH)
                nc.sync.dma_start(x8h_sb[:, :, :, qs], x8h_d[:, :, :, qs])
                nc.scalar.dma_start(x8l_sb[:, :, :, qs], x8l_d[:, :, :, qs])

            cos_sb = cpool.tile([P, S], f16, tag="cos")
            swap_sb = cpool.tile([P, S], f16, tag="swap")
            masks_sb = cpool.tile([P, P], bf, tag="masks")
            id64_sb = cpool.tile([HD, HD], f16, tag="id64")
            id128_sb = cpool.tile([P, P], bf, tag="id128")
            zbias = cpool.tile([P, 1], f32, tag="zbias")
            nc.gpsimd.memset(zbias[:], 0.0)
            nc.gpsimd.dma_start(cos_sb[:], cos_d[:])
            nc.gpsimd.dma_start(swap_sb[:], swap_d[:])
            nc.gpsimd.dma_start(masks_sb[:], masks_d[:])
            nc.gpsimd.dma_start(id64_sb[:], id64_d[:])
            nc.gpsimd.dma_start(id128_sb[:], id128_d[:])

            wot8h_sb = cpool.tile([P, 2, D], f8, tag="wot8h")
            wot8l_sb = cpool.tile([P, 2, D], f8, tag="wot8l")
            nc.gpsimd.dma_start(wot8h_sb[:], wot8h_d[:])
            nc.gpsimd.dma_start(wot8l_sb[:], wot8l_d[:])

            # q/k fp8 DoubleRow tiles: slot 0 written by rope, slot 1 dead
            # but must be exact zeros (it pairs with zeroed lhsT rows; and
            # junk fp8 bytes can be NaN)
            qt8 = [bigpool.tile([HD, 2, S], f8, tag=f"qt{h}", name=f"qt{h}") for h in range(QH)]
            kt8 = bigpool.tile([HD, 2, S], f8, tag="kt")
            for h in range(QH):
                nc.gpsimd.dma_start(qt8[h][:, 1, :], zeros_d[:])
            nc.gpsimd.dma_start(kt8[:, 1, :], zeros_d[:])

            vt_sb = bigpool.tile([HD, S], f16, tag="vt")
            yt_sb = [bigpool.tile([P, S], bf, tag=f"yt{m}", name=f"yt{m}") for m in range(2)]
            y8h_sb = bigpool.tile([P, 2, S], f8, tag="y8h")
            y8l_sb = bigpool.tile([P, 2, S], f8, tag="y8l")

            vaug_sb = [None] * NSQ

            # ---- projection chunk: 8 DoubleRow k-tiles x (xh*wh + xl*wh
            # + xh*wl), then RoPE straight into the fp8 DR tiles
            def proj_pass(ps, m, j, xi):
                chunk = slice(j * CH, (j + 1) * CH)
                mc = slice(m * P, (m + 1) * P)
                xs, ws = ((x8h_sb, w8h_sb), (x8l_sb, w8h_sb), (x8h_sb, w8l_sb))[xi]
                with nc.named_scope("proj"):
                    for kt in range(NKT):
                        nc.tensor.matmul(
                            ps[:],
                            ws[:, kt, :, mc],
                            xs[:, kt, :, chunk],
                            start=(kt == 0 and xi == 0),
                            stop=(kt == NKT - 1 and xi == 2),
                            perf_mode=DR,
                        )

            def proj_chunk(m, j):
                chunk = slice(j * CH, (j + 1) * CH)
                ps = ps_a.tile([P, CH], f32, tag="proj", name="ps_proj")
                # hi*hi pass first so the chain can start before the
                # lo-tensors finish loading
                for xi in range(3):
                    proj_pass(ps, m, j, xi)
                qr = tpool.tile([P, CH], f16, tag="rope_qr", name="rope_qr")
                with nc.named_scope("rope"):
                    nc.vector.tensor_copy(qr[:], ps[:])
                    t2 = tpool.tile([P, CH], f16, tag="rope_t2", name="rope_t2")
                    t3 = tpool.tile([P, CH], f16, tag="rope_t3", name="rope_t3")
                    if m < 2:
                        # rows: [R(h0) I(h0) R(h1) I(h1)] blocks of 32;
                        # out block R <- I-in * (-sin), I <- R-in * (+sin)
                        for b in range(4):
                            s0 = b * 32 + (32 if b % 2 == 0 else -32)
                            nc.gpsimd.tensor_mul(
                                t2[b * 32 : b * 32 + 32, :],
                                qr[s0 : s0 + 32, :],
                                swap_sb[s0 : s0 + 32, chunk],
                            )
                        nc.vector.tensor_mul(t3[:], qr[:], cos_sb[:, chunk])
                        for hh in range(2):
                            nc.gpsimd.tensor_add(
                                qt8[2 * m + hh][:, 0, chunk],
                                t3[hh * HD : (hh + 1) * HD, :],
                                t2[hh * HD : (hh + 1) * HD, :],
                            )
                    else:
                        # rows 0-63: [R(k) I(k)]; rows 64-127: v (natural)
                        nc.gpsimd.tensor_mul(t2[0:32, :], qr[32:64, :], swap_sb[32:64, chunk])
                        nc.gpsimd.tensor_mul(t2[32:64, :], qr[0:32, :], swap_sb[0:32, chunk])
                        nc.vector.tensor_mul(t3[0:HD, :], qr[0:HD, :], cos_sb[0:HD, chunk])
                        nc.gpsimd.tensor_add(kt8[:, 0, chunk], t3[0:HD, :], t2[0:HD, :])
                        nc.gpsimd.tensor_copy(vt_sb[:, chunk], qr[HD:P, :])

            ysb_tiles = {}

            # ---- scores + exp for one (chunk, head), weaving queued work
            # items (av chains / wo row-blocks) between score pairs so the
            # PE has filler while ACT chews through the exps
            def sdpa_scores(j, h, work):
                nlive = 4 * j + 4
                offs = [max(0, (i - 4 * j)) * P for i in range(nlive)]
                ets = []
                for i in range(0, nlive, 2):
                    with nc.named_scope("scores"):
                        ps2 = ps_s.tile([P, 2 * CH], f32, tag="sc", name="ps_sc")
                        o0, o1 = offs[i], offs[i + 1]
                        # u=1 output is shifted left by o1 so the pair's
                        # live region [o0, 2CH-o1) is contiguous and a
                        # single exp op covers it (av adjusts its indexing
                        # via the same shift)
                        nc.tensor.matmul(
                            ps2[:, o0:CH],
                            kt8[:, :, i * P : (i + 1) * P],
                            qt8[h][:, :, j * CH + o0 : (j + 1) * CH],
                            start=True,
                            stop=True,
                            perf_mode=DR,
                        )
                        nc.tensor.matmul(
                            ps2[:, CH : 2 * CH - o1],
                            kt8[:, :, (i + 1) * P : (i + 2) * P],
                            qt8[h][:, :, j * CH + o1 : (j + 1) * CH],
                            start=True,
                            stop=True,
                            perf_mode=DR,
                        )
                        et2 = epool.tile([P, 2 * CH], bf, tag="et", name="et")
                        with nc.named_scope("exp"):
                            nc.scalar.activation(
                                et2[:, o0 : 2 * CH - o1],
                                ps2[:, o0 : 2 * CH - o1],
                                mybir.ActivationFunctionType.Exp,
                                bias=zbias[:],
                                scale=0.125,
                            )
                        for u in range(2):
                            if i + u >= nlive - 4:  # diagonal tile: tri mask
                                off = u * CH + offs[i + u] - (o1 if u else 0)
                                with nc.named_scope("mask"):
                                    nc.gpsimd.tensor_mul(
                                        et2[:, off : off + P],
                                        et2[:, off : off + P],
                                        masks_sb[:],
                                    )
                        ets.append(et2)
                    pairs_left = (nlive - i) // 2 - 1
                    pops = 1
                    if pairs_left > 0:
                        pops = max(1, -(-len(work) // (pairs_left + 1)) - 1)
                    for _ in range(min(pops, len(work))):
                        work.pop(0)()
                return ets

            # ---- one flipped-AV chain + normalize for sq-tile t; the last
            # head also fires the tile's y transpose
            def av_tile(j, h, ets, t):
                tl = (t % 4) * P
                pav = ps_av.tile([P, HD + 1], f32, tag="av", name="ps_av")
                with nc.named_scope("av"):
                    for i in range(t + 1):
                        # odd tiles of a pair are stored shifted left by
                        # their causal offset (see sdpa_scores)
                        c0 = (i % 2) * CH + tl - ((i % 2) * max(0, i - 4 * j) * P)
                        nc.tensor.matmul(
                            pav[:],
                            ets[i // 2][:, c0 : c0 + P],
                            vaug_sb[i][:],
                            start=(i == 0),
                            stop=(i == t),
                        )
                with nc.named_scope("norm"):
                    recip = tpool.tile([P, 1], f32, tag="recip", name="recip")
                    nc.vector.reciprocal(recip[:], pav[:, HD : HD + 1])
                    ysb = ysb_tiles[t][h // 2]
                    nc.vector.tensor_scalar_mul(
                        ysb[:, (h % 2) * HD : (h % 2) * HD + HD],
                        pav[:, 0:HD],
                        recip[:],
                    )
                if h == QH - 1:
                    ytrans_tile(t, fast=(j == NCH - 1))

            def ytrans_tile(t, fast=False):
                # SBUF->SBUF DMA transpose (xbar), then Pool splits yT into
                # an fp8 hi+lo pair in DoubleRow layout for the fp8 wo.
                # fast=True (final chunk) uses a PE transpose + DVE drain
                # instead: ~0.5us latency vs ~2.4us for the DMA path.
                ts_ = slice(t * P, (t + 1) * P)
                with nc.named_scope("ytrans"):
                    for k in range(2):
                        if fast:
                            pt = ps_av.tile([P, P], bf, tag="av", name="ps_yt")
                            nc.tensor.transpose(pt[:], ysb_tiles[t][k][:], id128_sb[:])
                            nc.vector.tensor_copy(yt_sb[k][:, ts_], pt[:])
                        else:
                            nc.sync.dma_start(
                                yt_sb[k][:, ts_], ysb_tiles[t][k][:], transpose=True
                            )
                        nc.gpsimd.tensor_copy(y8h_sb[:, k, ts_], yt_sb[k][:, ts_])
                        nc.vector.scalar_tensor_tensor(
                            y8l_sb[:, k, ts_],
                            yt_sb[k][:, ts_],
                            1.0,
                            y8h_sb[:, k, ts_],
                            mybir.AluOpType.mult,
                            mybir.AluOpType.subtract,
                        )

            ot_live = {}

            def wo_dch(sm, dcJ):
                # one [128, 512] block of the output projection: fp8
                # DoubleRow hi+lo (y*w ~= yh*wh + yl*wh + yh*wl); the drain
                # folds in the 1/32 fp8 weight scale
                srow = slice(sm * P, (sm + 1) * P)
                if dcJ == 0:
                    ot_live[sm] = opool.tile([P, D], f16, tag="ot", name="ot")
                ot = ot_live[sm]
                dch = slice(dcJ * CH, (dcJ + 1) * CH)
                pw = ps_a.tile([P, CH], f32, tag="proj", name="ps_wo")
                with nc.named_scope("wo"):
                    for wi, (ys, ws) in enumerate(
                        ((y8h_sb, wot8h_sb), (y8l_sb, wot8h_sb), (y8h_sb, wot8l_sb))
                    ):
                        nc.tensor.matmul(
                            pw[:],
                            ys[:, :, srow],
                            ws[:, :, dch],
                            start=(wi == 0),
                            stop=(wi == 2),
                            perf_mode=DR,
                        )
                with nc.named_scope("outdrain"):
                    # in the final chunk ACT is idle; alternating drains
                    # doubles the drain rate that throttles the tail
                    if sm >= 4 * (NCH - 1) and dcJ % 2 == 0:
                        nc.scalar.activation(
                            ot[:, dch],
                            pw[:],
                            mybir.ActivationFunctionType.Copy,
                            scale=1.0 / WSCALE,
                        )
                    else:
                        nc.vector.tensor_scalar_mul(ot[:, dch], pw[:], 1.0 / WSCALE)
                if sm >= 4 * (NCH - 1):
                    # final chunk: ship each dch block as soon as it drains
                    with nc.named_scope("outdma"):
                        nc.sync.dma_start(out_d[srow, dch], ot[:, dch])
                        if dcJ == NCH - 1:
                            del ot_live[sm]
                elif dcJ == NCH - 1:
                    with nc.named_scope("outdma"):
                        nc.sync.dma_start(out_d[srow, :], ot[:])
                        del ot_live[sm]

            def wo_tile(sm):
                for dcJ in range(NCH):
                    wo_dch(sm, dcJ)

            def vtrans_chunk(j):
                with nc.named_scope("vtrans"):
                    for i in range(4 * j, 4 * j + 4):
                        va = vpool.tile([P, HD + 1], f16, tag="vaug", name=f"vaug{i}")
                        nc.sync.dma_start(
                            va[:, 0:HD], vt_sb[:, i * P : (i + 1) * P], transpose=True
                        )
                        nc.gpsimd.memset(va[:, HD : HD + 1], WSCALE)
                        vaug_sb[i] = va

            # ---- emission: ascending chunks. Everything except scores
            # (projections for the NEXT chunk, av chains of the previous
            # head, wo row-blocks) rides in a FIFO of work items that
            # sdpa_scores pops between score pairs, so the PE always has
            # filler while ACT chews exps; any leftovers are force-drained
            # right after a chunk's last scores (while its exps still run).
            proj_chunk(2, 0)
            vtrans_chunk(0)
            proj_chunk(0, 0)
            proj_chunk(1, 0)
            prev = None  # (j, h, ets)
            work = []
            urgent = []
            pending_wo = []
            seq = [0, 1, 2, 3]
            # per slot (j, h): projection work for the next chunk
            projq = {}
            for jj in range(NCH - 1):
                projq[(jj, 0)] = [
                    lambda jj=jj: proj_chunk(2, jj + 1),
                    lambda jj=jj: vtrans_chunk(jj + 1),
                ]
                projq[(jj, 1)] = [lambda jj=jj: proj_chunk(0, jj + 1)]
                projq[(jj, 2)] = [lambda jj=jj: proj_chunk(1, jj + 1)]
            for j in seq:
                for t in range(4 * j, 4 * j + 4):
                    ysb_tiles[t] = [
                        ypool.tile([P, P], bf, tag=f"ysb{k}", name=f"ysb{k}_{t}")
                        for k in range(2)
                    ]
                for h in range(QH):
                    work.extend(pending_wo)
                    pending_wo = []
                    if prev is not None:
                        pj, ph, pets = prev
                        for t in range(4 * pj, 4 * pj + 4):
                            work.append(
                                lambda pj=pj, ph=ph, pets=pets, t=t: av_tile(
                                    pj, ph, pets, t
                                )
                            )
                        if ph == QH - 1:
                            # one-slot delay so the ytrans+fp8-cast chain
                            # completes before the wo matmuls want it
                            pending_wo = [
                                (lambda t=t, d=d: wo_dch(t, d))
                                for t in range(4 * pj, 4 * pj + 4)
                                for d in range(NCH)
                            ]
                    for it in reversed(projq.get((j, h), [])):
                        work.insert(0, it)
                    ets = sdpa_scores(j, h, work)
                    prev = (j, h, ets)
                # force any not-yet-popped next-chunk projections (and
                # stragglers) out now, under this chunk's trailing exps
                while work:
                    work.pop(0)()
            # final head: per-tile av/norm/ytrans then wo
            while work:
                work.pop(0)()
            for w in pending_wo:
                w()
            pending_wo = []
            fj, fh, fets = prev
            for t in range(4 * fj, 4 * fj + 4):
                av_tile(fj, fh, fets, t)
                if t > 4 * fj:
                    wo_tile(t - 1)
            wo_tile(4 * fj + 3)

    nc.finalize()
    return nc


def _host_inputs(x, freqs_cos, freqs_sin, wq, wk, wv, wo):
    """Build the 8 per-core input maps (all host-side preprocessing)."""
    x = np.asarray(x, np.float32)
    cos = np.asarray(freqs_cos, np.float32)  # [S, 32]
    sin = np.asarray(freqs_sin, np.float32)
    wq = np.asarray(wq, np.float32)
    wk = np.asarray(wk, np.float32)
    wv = np.asarray(wv, np.float32)
    wo = np.asarray(wo, np.float32)

    # de-interleave pairs into [R | I] blocks of 32 within each head
    perm = np.concatenate([np.arange(0, HD, 2), np.arange(1, HD, 2)])

    xt = np.ascontiguousarray(x[0].T)  # [D, S]

    def dr8(a):
        """[D, cols] -> hi/lo fp8 pair in DoubleRow layout [P, NKT, 2, cols]."""
        hi = a.astype(F8)
        lo = (a - hi.astype(np.float32)).astype(F8)
        out = []
        for t in (hi, lo):
            out.append(
                np.ascontiguousarray(
                    t.reshape(NKT, 2, P, -1).transpose(2, 0, 1, 3)
                )
            )
        return out

    x8h, x8l = dr8(xt)

    # trig tables with the 1/32 fp8 weight scale folded in;
    # rows (mod 64): [ +sin (R rows, feeds I-out) is NOT how the mul reads:
    # table row follows the INPUT partition: rows 0-31 (R in) -> +sin,
    # rows 32-63 (I in) -> -sin ]; cos everywhere.
    cos128 = np.empty((P, S), np.float16)
    swap128 = np.empty((P, S), np.float16)
    for dd in range(P):
        i = dd % 32
        cos128[dd] = cos[:, i] / WSCALE
        swap128[dd] = (sin[:, i] if (dd % HD) < 32 else -sin[:, i]) / WSCALE

    pp = np.arange(P)[:, None]
    ff = np.arange(P)[None, :]
    masks = (pp <= ff).astype(np.float32).astype(BF16)
    zeros8 = np.zeros((HD, S), F8)
    ident64 = np.eye(HD, dtype=np.float16)
    ident128 = np.eye(P, dtype=np.float32).astype(BF16)

    in_maps = []
    for c in range(NCORES):
        wq_c = wq[c * QCOLS : (c + 1) * QCOLS].reshape(QH, HD, D)[:, perm, :].reshape(
            QCOLS, D
        )
        wk_c = wk[c * HD : (c + 1) * HD][perm, :]
        wv_c = wv[c * HD : (c + 1) * HD]
        wqkvt = np.ascontiguousarray(
            np.concatenate([wq_c, wk_c, wv_c], axis=0).T
        ) * WSCALE  # [D, COLS]
        w8h, w8l = dr8(wqkvt)
        # wo.T in DoubleRow layout [P, 2, D] (slot = head-pair), x32 scale
        wot = np.ascontiguousarray(
            wo[:, c * QCOLS : (c + 1) * QCOLS].T.reshape(2, P, D).transpose(1, 0, 2)
        ) * WSCALE
        wot8h = wot.astype(F8)
        wot8l = (wot - wot8h.astype(np.float32)).astype(F8)
        in_maps.append(
            {
                "x8h": x8h,
                "x8l": x8l,
                "w8h": w8h,
                "w8l": w8l,
                "wot8h": wot8h,
                "wot8l": wot8l,
                "cos128": cos128,
                "swap128": swap128,
                "masks": masks,
                "zeros8": zeros8,
                "ident64": ident64,
                "ident128": ident128,
            }
        )
    return in_maps


def kernel(x, freqs_cos, freqs_sin, wq, wk, wv, wo):
    from concourse.bass_utils import run_bass_kernel_spmd

    if "nc" not in _CACHE:
        _CACHE["nc"] = _build()
    nc = _CACHE["nc"]
    in_maps = _host_inputs(x, freqs_cos, freqs_sin, wq, wk, wv, wo)
    res = run_bass_kernel_spmd(nc, in_maps, core_ids=list(range(NCORES)))
    out = np.zeros((S, D), np.float64)
    for r in res.results:
        out += r["out"].astype(np.float64)
    return out.astype(np.float32).reshape(1, S, D)


# revision 112
# speedup vs baseline: 1.0103x; 1.0078x over previous
"""GQA causal attention (llama3-style RoPE) on 8 TRN2 NeuronCores.

Sharding: tensor-parallel over heads. Core c gets q-heads 4c..4c+3 and
kv-head c (GQA groups intact), plus the matching row-block of wo.T.
Each core computes a full [S, D] partial of the output projection; the
host sums the 8 partials (the "all-reduce" of the row-sharded wo).

Per-core pipeline:
  qkvT = wqkvT.T @ xT        fp8e4 DoubleRow (K=256/tile) with hi+lo
                             error compensation: x*w ~= xh*wh + xl*wh
                             + xh*wl -> ~bf16 accuracy at 0.75x the PE
                             cycles of a bf16 matmul chain.
  RoPE on qT/kT (de-interleaved [R|I] head layout via host-permuted
                             weight cols; 1/32 weight scale folded into
                             the trig tables), final add writes fp8
                             directly into DoubleRow layout [64,2,S]
                             (dead slot 1 zeroed once; K padded 64->128)
  sT[sk, sq] = kT.T @ qT     fp8e4 DoubleRow: 0.5 cyc/col
  eT = exp(sT/8) * mask      ACT, psum->bf16 SBUF
  avT[sq, 65] = eT.T @ vaug  et stationary, v_aug moving: 65 cols/tile
                             instead of 512 (cost model charges moving
                             cols only). vaug col 64 = 32.0 (folds the
                             1/32 v descale into the softmax denom).
  y = avT[:, 0:64] * recip   DVE recip + Pool per-partition scale
  yT via DMA-transpose       (SBUF xbar, no PE/psum cost)
  out[sq, d] = yT.T @ woT    bf16; drains DVE/Pool -> f16; host sums.
"""

import sys

for _p in ("/opt/trn_rl_repo", "/root/.axon_site/_ro/trn_rl_repo"):
    if _p not in sys.path:
        sys.path.insert(0, _p)

import numpy as np
import ml_dtypes

import concourse.bass as bass
import concourse.bacc as bacc
import concourse.mybir as mybir
import concourse.tile as tile

BF16 = ml_dtypes.bfloat16
F8 = ml_dtypes.float8_e4m3

S = 2048
D = 2048
HD = 64
NH = 32
NKV = 8
NCORES = 8
QH = NH // NCORES            # 4 local q heads
QCOLS = QH * HD              # 256
KVCOLS = 2 * HD              # 128 (k and v, one kv head)
COLS = QCOLS + KVCOLS        # 384
P = 128                      # partitions
NKT = 8                      # DoubleRow contraction tiles (K=256 each)
NSQ = S // P                 # 16 seq tiles of 128
NCH = 4                      # seq chunks of 512
CH = 512
WSCALE = 32.0                # fp8 range scale on wq/wk/wv

_CACHE = {}


def _build():
    f8 = mybir.dt.float8e4
    bf = mybir.dt.bfloat16
    f16 = mybir.dt.float16
    f32 = mybir.dt.float32
    DR = mybir.MatmulPerfMode.DoubleRow

    nc = bacc.Bacc()
    x8h_d = nc.dram_tensor("x8h", [P, NKT, 2, S], f8, kind="ExternalInput")
    x8l_d = nc.dram_tensor("x8l", [P, NKT, 2, S], f8, kind="ExternalInput")
    w8h_d = nc.dram_tensor("w8h", [P, NKT, 2, COLS], f8, kind="ExternalInput")
    w8l_d = nc.dram_tensor("w8l", [P, NKT, 2, COLS], f8, kind="ExternalInput")
    wot8h_d = nc.dram_tensor("wot8h", [P, 2, D], f8, kind="ExternalInput")
    wot8l_d = nc.dram_tensor("wot8l", [P, 2, D], f8, kind="ExternalInput")
    cos_d = nc.dram_tensor("cos128", [P, S], f16, kind="ExternalInput")
    swap_d = nc.dram_tensor("swap128", [P, S], f16, kind="ExternalInput")
    masks_d = nc.dram_tensor("masks", [P, P], bf, kind="ExternalInput")
    zeros_d = nc.dram_tensor("zeros8", [HD, S], f8, kind="ExternalInput")
    id64_d = nc.dram_tensor("ident64", [HD, HD], f16, kind="ExternalInput")
    id128_d = nc.dram_tensor("ident128", [P, P], bf, kind="ExternalInput")
    out_d = nc.dram_tensor("out", [S, D], f16, kind="ExternalOutput")

    with tile.TileContext(nc) as tc:
        with (
            tc.tile_pool(name="const", bufs=1) as cpool,
            tc.tile_pool(name="x8", bufs=1) as xpool,
            tc.tile_pool(name="w8", bufs=1) as wpool,
            tc.tile_pool(name="big", bufs=1) as bigpool,
            tc.tile_pool(name="vaug", bufs=NSQ) as vpool,
            tc.tile_pool(name="et", bufs=18) as epool,
            tc.tile_pool(name="tmp", bufs=3) as tpool,
            tc.tile_pool(name="ysb", bufs=4) as ypool,
            tc.tile_pool(name="ot", bufs=2) as opool,
            tc.tile_pool(name="ps_a", bufs=2, space="PSUM") as ps_a,
            tc.tile_pool(name="ps_s", bufs=2, space="PSUM") as ps_s,
            tc.tile_pool(name="ps_av", bufs=2, space="PSUM") as ps_av,
        ):
            # proj-critical loads first on the serial DMA resource: weights,
            # then x in chunk-quarters so proj chunk j starts when quarter
            # j lands; small tables (needed later) afterwards via SWDGE
            w8h_sb = wpool.tile([P, NKT, 2, COLS], f8, tag="w8h")
            w8l_sb = wpool.tile([P, NKT, 2, COLS], f8, tag="w8l")
            x8h_sb = xpool.tile([P, NKT, 2, S], f8, tag="x8h")
            x8l_sb = xpool.tile([P, NKT, 2, S], f8, tag="x8l")
            # emission order = serial DMA-resource order; front-load exactly
            # what the first (kv, chunk 0) projection pass needs
            nc.sync.dma_start(w8h_sb[:, :, :, 2 * P :], w8h_d[:, :, :, 2 * P :])
            nc.sync.dma_start(x8h_sb[:, 0:4, :, 0:CH], x8h_d[:, 0:4, :, 0:CH])
            nc.sync.dma_start(x8h_sb[:, 4:8, :, 0:CH], x8h_d[:, 4:8, :, 0:CH])
            nc.scalar.dma_start(x8l_sb[:, 0:4, :, 0:CH], x8l_d[:, 0:4, :, 0:CH])
            nc.scalar.dma_start(x8l_sb[:, 4:8, :, 0:CH], x8l_d[:, 4:8, :, 0:CH])
            nc.scalar.dma_start(w8l_sb[:, :, :, 2 * P :], w8l_d[:, :, :, 2 * P :])
            nc.sync.dma_start(w8h_sb[:, :, :, : 2 * P], w8h_d[:, :, :, : 2 * P])
            nc.scalar.dma_start(w8l_sb[:, :, :, : 2 * P], w8l_d[:, :, :, : 2 * P])
            for q in range(1, NCH):
                qs = slice(q * CH, (q + 1) * CH)
                nc.sync.dma_start(x8h_sb[:, :, :, qs], x8h_d[:, :, :, qs])
                nc.scalar.dma_start(x8l_sb[:, :, :, qs], x8l_d[:, :, :, qs])

            cos_sb = cpool.tile([P, S], f16, tag="cos")
            swap_sb = cpool.tile([P, S], f16, tag="swap")
            masks_sb = cpool.tile([P, P], bf, tag="masks")
            id64_sb = cpool.tile([HD, HD], f16, tag="id64")
            id128_sb = cpool.tile([P, P], bf, tag="id128")
            zbias = cpool.tile([P, 1], f32, tag="zbias")
            nc.gpsimd.memset(zbias[:], 0.0)
            nc.gpsimd.dma_start(cos_sb[:], cos_d[:])
            nc.gpsimd.dma_start(swap_sb[:], swap_d[:])
            nc.gpsimd.dma_start(masks_sb[:], masks_d[:])
            nc.gpsimd.dma_start(id64_sb[:], id64_d[:])
            nc.gpsimd.dma_start(id128_sb[:], id128_d[:])

            wot8h_sb = cpool.tile([P, 2, D], f8, tag="wot8h")
            wot8l_sb = cpool.tile([P, 2, D], f8, tag="wot8l")
            nc.gpsimd.dma_start(wot8h_sb[:], wot8h_d[:])
            nc.gpsimd.dma_start(wot8l_sb[:], wot8l_d[:])

            # q/k fp8 DoubleRow tiles: slot 0 written by rope, slot 1 dead
            # but must be exact zeros (it pairs with zeroed lhsT rows; and
            # junk fp8 bytes can be NaN)
            qt8 = [bigpool.tile([HD, 2, S], f8, tag=f"qt{h}", name=f"qt{h}") for h in range(QH)]
            kt8 = bigpool.tile([HD, 2, S], f8, tag="kt")
            for h in range(QH):
                nc.gpsimd.dma_start(qt8[h][:, 1, :], zeros_d[:])
            nc.gpsimd.dma_start(kt8[:, 1, :], zeros_d[:])

            vt_sb = bigpool.tile([HD, S], f16, tag="vt")
            yt_sb = [bigpool.tile([P, S], bf, tag=f"yt{m}", name=f"yt{m}") for m in range(2)]
            y8h_sb = bigpool.tile([P, 2, S], f8, tag="y8h")
            y8l_sb = bigpool.tile([P, 2, S], f8, tag="y8l")

            vaug_sb = [None] * NSQ

            # ---- projection chunk: 8 DoubleRow k-tiles x (xh*wh + xl*wh
            # + xh*wl), then RoPE straight into the fp8 DR tiles
            def proj_pass(ps, m, j, xi):
                chunk = slice(j * CH, (j + 1) * CH)
                mc = slice(m * P, (m + 1) * P)
                xs, ws = ((x8h_sb, w8h_sb), (x8l_sb, w8h_sb), (x8h_sb, w8l_sb))[xi]
                with nc.named_scope("proj"):
                    for kt in range(NKT):
                        nc.tensor.matmul(
                            ps[:],
                            ws[:, kt, :, mc],
                            xs[:, kt, :, chunk],
                            start=(kt == 0 and xi == 0),
                            stop=(kt == NKT - 1 and xi == 2),
                            perf_mode=DR,
                        )

            def proj_chunk(m, j):
                chunk = slice(j * CH, (j + 1) * CH)
                ps = ps_a.tile([P, CH], f32, tag="proj", name="ps_proj")
                # hi*hi pass first so the chain can start before the
                # lo-tensors finish loading
                for xi in range(3):
                    proj_pass(ps, m, j, xi)
                qr = tpool.tile([P, CH], f16, tag="rope_qr", name="rope_qr")
                with nc.named_scope("rope"):
                    nc.vector.tensor_copy(qr[:], ps[:])
                    t2 = tpool.tile([P, CH], f16, tag="rope_t2", name="rope_t2")
                    t3 = tpool.tile([P, CH], f16, tag="rope_t3", name="rope_t3")
                    if m < 2:
                        # rows: [R(h0) I(h0) R(h1) I(h1)] blocks of 32;
                        # out block R <- I-in * (-sin), I <- R-in * (+sin)
                        for b in range(4):
                            s0 = b * 32 + (32 if b % 2 == 0 else -32)
                            nc.gpsimd.tensor_mul(
                                t2[b * 32 : b * 32 + 32, :],
                                qr[s0 : s0 + 32, :],
                                swap_sb[s0 : s0 + 32, chunk],
                            )
                        nc.vector.tensor_mul(t3[:], qr[:], cos_sb[:, chunk])
                        for hh in range(2):
                            nc.gpsimd.tensor_add(
                                qt8[2 * m + hh][:, 0, chunk],
                                t3[hh * HD : (hh + 1) * HD, :],
                                t2[hh * HD : (hh + 1) * HD, :],
                            )
                    else:
                        # rows 0-63: [R(k) I(k)]; rows 64-127: v (natural)
                        nc.gpsimd.tensor_mul(t2[0:32, :], qr[32:64, :], swap_sb[32:64, chunk])
                        nc.gpsimd.tensor_mul(t2[32:64, :], qr[0:32, :], swap_sb[0:32, chunk])
                        nc.vector.tensor_mul(t3[0:HD, :], qr[0:HD, :], cos_sb[0:HD, chunk])
                        nc.gpsimd.tensor_add(kt8[:, 0, chunk], t3[0:HD, :], t2[0:HD, :])
                        nc.gpsimd.tensor_copy(vt_sb[:, chunk], qr[HD:P, :])

            ysb_tiles = {}

            # ---- scores + exp for one (chunk, head), weaving queued work
            # items (av chains / wo row-blocks) between score pairs so the
            # PE has filler while ACT chews through the exps
            def sdpa_scores(j, h, work):
                nlive = 4 * j + 4
                offs = [max(0, (i - 4 * j)) * P for i in range(nlive)]
                ets = []
                for i in range(0, nlive, 2):
                    with nc.named_scope("scores"):
                        ps2 = ps_s.tile([P, 2 * CH], f32, tag="sc", name="ps_sc")
                        o0, o1 = offs[i], offs[i + 1]
                        # u=1 output is shifted left by o1 so the pair's
                        # live region [o0, 2CH-o1) is contiguous and a
                        # single exp op covers it (av adjusts its indexing
                        # via the same shift)
                        nc.tensor.matmul(
                            ps2[:, o0:CH],
                            kt8[:, :, i * P : (i + 1) * P],
                            qt8[h][:, :, j * CH + o0 : (j + 1) * CH],
                            start=True,
                            stop=True,
                            perf_mode=DR,
                        )
                        nc.tensor.matmul(
                            ps2[:, CH : 2 * CH - o1],
                            kt8[:, :, (i + 1) * P : (i + 2) * P],
                            qt8[h][:, :, j * CH + o1 : (j + 1) * CH],
                            start=True,
                            stop=True,
                            perf_mode=DR,
                        )
                        et2 = epool.tile([P, 2 * CH], bf, tag="et", name="et")
                        with nc.named_scope("exp"):
                            nc.scalar.activation(
                                et2[:, o0 : 2 * CH - o1],
                                ps2[:, o0 : 2 * CH - o1],
                                mybir.ActivationFunctionType.Exp,
                                bias=zbias[:],
                                scale=0.125,
                            )
                        for u in range(2):
                            if i + u >= nlive - 4:  # diagonal tile: tri mask
                                off = u * CH + offs[i + u] - (o1 if u else 0)
                                with nc.named_scope("mask"):
                                    nc.gpsimd.tensor_mul(
                                        et2[:, off : off + P],
                                        et2[:, off : off + P],
                                        masks_sb[:],
                                    )
                        ets.append(et2)
                    pairs_left = (nlive - i) // 2 - 1
                    pops = 1
                    if pairs_left > 0:
                        pops = max(1, -(-len(work) // (pairs_left + 1)) - 1)
                    for _ in range(min(pops, len(work))):
                        work.pop(0)()
                return ets

            # ---- one flipped-AV chain + normalize for sq-tile t; the last
            # head also fires the tile's y transpose
            def av_tile(j, h, ets, t):
                tl = (t % 4) * P
                pav = ps_av.tile([P, HD + 1], f32, tag="av", name="ps_av")
                with nc.named_scope("av"):
                    for i in range(t + 1):
                        # odd tiles of a pair are stored shifted left by
                        # their causal offset (see sdpa_scores)
                        c0 = (i % 2) * CH + tl - ((i % 2) * max(0, i - 4 * j) * P)
                        nc.tensor.matmul(
                            pav[:],
                            ets[i // 2][:, c0 : c0 + P],
                            vaug_sb[i][:],
                            start=(i == 0),
                            stop=(i == t),
                        )
                with nc.named_scope("norm"):
                    recip = tpool.tile([P, 1], f32, tag="recip", name="recip")
                    nc.vector.reciprocal(recip[:], pav[:, HD : HD + 1])
                    ysb = ysb_tiles[t][h // 2]
                    nc.vector.tensor_scalar_mul(
                        ysb[:, (h % 2) * HD : (h % 2) * HD + HD],
                        pav[:, 0:HD],
                        recip[:],
                    )
                if h == QH - 1:
                    ytrans_tile(t, fast=(j == NCH - 1))

            def ytrans_tile(t, fast=False):
                # SBUF->SBUF DMA transpose (xbar), then Pool splits yT into
                # an fp8 hi+lo pair in DoubleRow layout for the fp8 wo.
                # fast=True (final chunk) uses a PE transpose + DVE drain
                # instead: ~0.5us latency vs ~2.4us for the DMA path.
                ts_ = slice(t * P, (t + 1) * P)
                with nc.named_scope("ytrans"):
                    for k in range(2):
                        if fast:
                            pt = ps_av.tile([P, P], bf, tag="av", name="ps_yt")
                            nc.tensor.transpose(pt[:], ysb_tiles[t][k][:], id128_sb[:])
                            nc.vector.tensor_copy(yt_sb[k][:, ts_], pt[:])
                        else:
                            nc.sync.dma_start(
                                yt_sb[k][:, ts_], ysb_tiles[t][k][:], transpose=True
                            )
                        nc.gpsimd.tensor_copy(y8h_sb[:, k, ts_], yt_sb[k][:, ts_])
                        nc.vector.scalar_tensor_tensor(
                            y8l_sb[:, k, ts_],
                            yt_sb[k][:, ts_],
                            1.0,
                            y8h_sb[:, k, ts_],
                            mybir.AluOpType.mult,
                            mybir.AluOpType.subtract,
                        )

            ot_live = {}

            def wo_dch(sm, dcJ):
                # one [128, 512] block of the output projection: fp8
                # DoubleRow hi+lo (y*w ~= yh*wh + yl*wh + yh*wl); the drain
                # folds in the 1/32 fp8 weight scale
                srow = slice(sm * P, (sm + 1) * P)
                if dcJ == 0:
                    ot_live[sm] = opool.tile([P, D], f16, tag="ot", name="ot")
                ot = ot_live[sm]
                dch = slice(dcJ * CH, (dcJ + 1) * CH)
                pw = ps_a.tile([P, CH], f32, tag="proj", name="ps_wo")
                with nc.named_scope("wo"):
                    for wi, (ys, ws) in enumerate(
                        ((y8h_sb, wot8h_sb), (y8l_sb, wot8h_sb), (y8h_sb, wot8l_sb))
                    ):
                        nc.tensor.matmul(
                            pw[:],
                            ys[:, :, srow],
                            ws[:, :, dch],
                            start=(wi == 0),
                            stop=(wi == 2),
                            perf_mode=DR,
                        )
                with nc.named_scope("outdrain"):
                    # in the final chunk ACT is idle; alternating drains
                    # doubles the drain rate that throttles the tail
                    if sm >= 4 * (NCH - 1) and dcJ % 2 == 0:
                        nc.scalar.activation(
                            ot[:, dch],
                            pw[:],
                            mybir.ActivationFunctionType.Copy,
                            scale=1.0 / WSCALE,
                        )
                    else:
                        nc.vector.tensor_scalar_mul(ot[:, dch], pw[:], 1.0 / WSCALE)
                if sm >= 4 * (NCH - 1):
                    # final chunk: ship each dch block as soon as it drains
                    with nc.named_scope("outdma"):
                        nc.sync.dma_start(out_d[srow, dch], ot[:, dch])
                        if dcJ == NCH - 1:
                            del ot_live[sm]
                elif dcJ == NCH - 1:
                    with nc.named_scope("outdma"):
                        nc.sync.dma_start(out_d[srow, :], ot[:])
                        del ot_live[sm]

            def wo_tile(sm):
                for dcJ in range(NCH):
                    wo_dch(sm, dcJ)

            def vtrans_chunk(j):
                with nc.named_scope("vtrans"):
                    for i in range(4 * j, 4 * j + 4):
                        va = vpool.tile([P, HD + 1], f16, tag="vaug", name=f"vaug{i}")
                        nc.sync.dma_start(
                            va[:, 0:HD], vt_sb[:, i * P : (i + 1) * P], transpose=True
                        )
                        nc.gpsimd.memset(va[:, HD : HD + 1], WSCALE)
                        vaug_sb[i] = va

            # ---- emission: ascending chunks. Everything except scores
            # (projections for the NEXT chunk, av chains of the previous
            # head, wo row-blocks) rides in a FIFO of work items that
            # sdpa_scores pops between score pairs, so the PE always has
            # filler while ACT chews exps; any leftovers are force-drained
            # right after a chunk's last scores (while its exps still run).
            proj_chunk(2, 0)
            vtrans_chunk(0)
            proj_chunk(0, 0)
            proj_chunk(1, 0)
            prev = None  # (j, h, ets)
            work = []
            urgent = []
            pending_wo = []
            seq = [0, 1, 2, 3]
            # per slot (j, h): projection work for the next chunk
            projq = {}
            for jj in range(NCH - 1):
                projq[(jj, 0)] = [
                    lambda jj=jj: proj_chunk(2, jj + 1),
                    lambda jj=jj: vtrans_chunk(jj + 1),
                ]
                projq[(jj, 1)] = [lambda jj=jj: proj_chunk(0, jj + 1)]
                projq[(jj, 2)] = [lambda jj=jj: proj_chunk(1, jj + 1)]
            for j in seq:
                for t in range(4 * j, 4 * j + 4):
                    ysb_tiles[t] = [
                        ypool.tile([P, P], bf, tag=f"ysb{k}", name=f"ysb{k}_{t}")
                        for k in range(2)
                    ]
                for h in range(QH):
                    work.extend(pending_wo)
                    pending_wo = []
                    if prev is not None:
                        pj, ph, pets = prev
                        for t in range(4 * pj, 4 * pj + 4):
                            work.append(
                                lambda pj=pj, ph=ph, pets=pets, t=t: av_tile(
                                    pj, ph, pets, t
                                )
                            )
                        if ph == QH - 1:
                            # one-slot delay so the ytrans+fp8-cast chain
                            # completes before the wo matmuls want it
                            pending_wo = [
                                (lambda t=t, d=d: wo_dch(t, d))
                                for t in range(4 * pj, 4 * pj + 4)
                                for d in range(NCH)
                            ]
                    for it in reversed(projq.get((j, h), [])):
                        work.insert(0, it)
                    ets = sdpa_scores(j, h, work)
                    prev = (j, h, ets)
                # force any not-yet-popped next-chunk projections (and
                # stragglers) out now, under this chunk's trailing exps
                while work:
                    work.pop(0)()
            # final head: per-tile av/norm/ytrans then wo
            while work:
                work.pop(0)()
            for w in pending_wo:
                w()
            pending_wo = []
            fj, fh, fets = prev
            for t in range(4 * fj, 4 * fj + 4):
                av_tile(fj, fh, fets, t)
                if t > 4 * fj:
                    wo_tile(t - 1)
            wo_tile(4 * fj + 3)

    nc.finalize()
    return nc


def _host_inputs(x, freqs_cos, freqs_sin, wq, wk, wv, wo):
    """Build the 8 per-core input maps (all host-side preprocessing)."""
    x = np.asarray(x, np.float32)
    cos = np.asarray(freqs_cos, np.float32)  # [S, 32]
    sin = np.asarray(freqs_sin, np.float32)
    wq = np.asarray(wq, np.float32)
    wk = np.asarray(wk, np.float32)
    wv = np.asarray(wv, np.float32)
    wo = np.asarray(wo, np.float32)

    # de-interleave pairs into [R | I] blocks of 32 within each head
    perm = np.concatenate([np.arange(0, HD, 2), np.arange(1, HD, 2)])

    xt = np.ascontiguousarray(x[0].T)  # [D, S]

    def dr8(a):
        """[D, cols] -> hi/lo fp8 pair in DoubleRow layout [P, NKT, 2, cols]."""
        hi = a.astype(F8)
        lo = (a - hi.astype(np.float32)).astype(F8)
        out = []
        for t in (hi, lo):
            out.append(
                np.ascontiguousarray(
                    t.reshape(NKT, 2, P, -1).transpose(2, 0, 1, 3)
                )
            )
        return out

    x8h, x8l = dr8(xt)

    # trig tables with the 1/32 fp8 weight scale folded in;
    # rows (mod 64): [ +sin (R rows, feeds I-out) is NOT how the mul reads:
    # table row follows the INPUT partition: rows 0-31 (R in) -> +sin,
    # rows 32-63 (I in) -> -sin ]; cos everywhere.
    cos128 = np.empty((P, S), np.float16)
    swap128 = np.empty((P, S), np.float16)
    for dd in range(P):
        i = dd % 32
        cos128[dd] = cos[:, i] / WSCALE
        swap128[dd] = (sin[:, i] if (dd % HD) < 32 else -sin[:, i]) / WSCALE

    pp = np.arange(P)[:, None]
    ff = np.arange(P)[None, :]
    masks = (pp <= ff).astype(np.float32).astype(BF16)
    zeros8 = np.zeros((HD, S), F8)
    ident64 = np.eye(HD, dtype=np.float16)
    ident128 = np.eye(P, dtype=np.float32).astype(BF16)

    in_maps = []
    for c in range(NCORES):
        wq_c = wq[c * QCOLS : (c + 1) * QCOLS].reshape(QH, HD, D)[:, perm, :].reshape(
            QCOLS, D
        )
        wk_c = wk[c * HD : (c + 1) * HD][perm, :]
        wv_c = wv[c * HD : (c + 1) * HD]
        wqkvt = np.ascontiguousarray(
            np.concatenate([wq_c, wk_c, wv_c], axis=0).T
        ) * WSCALE  # [D, COLS]
        w8h, w8l = dr8(wqkvt)
        # wo.T in DoubleRow layout [P, 2, D] (slot = head-pair), x32 scale
        wot = np.ascontiguousarray(
            wo[:, c * QCOLS : (c + 1) * QCOLS].T.reshape(2, P, D).transpose(1, 0, 2)
        ) * WSCALE
        wot8h = wot.astype(F8)
        wot8l = (wot - wot8h.astype(np.float32)).astype(F8)
        in_maps.append(
            {
                "x8h": x8h,
                "x8l": x8l,
                "w8h": w8h,
                "w8l": w8l,
                "wot8h": wot8h,
                "wot8l": wot8l,
                "cos128": cos128,
                "swap128": swap128,
                "masks": masks,
                "zeros8": zeros8,
                "ident64": ident64,
                "ident128": ident128,
            }
        )
    return in_maps


def kernel(x, freqs_cos, freqs_sin, wq, wk, wv, wo):
    from concourse.bass_utils import run_bass_kernel_spmd

    if "nc" not in _CACHE:
        _CACHE["nc"] = _build()
    nc = _CACHE["nc"]
    in_maps = _host_inputs(x, freqs_cos, freqs_sin, wq, wk, wv, wo)
    res = run_bass_kernel_spmd(nc, in_maps, core_ids=list(range(NCORES)))
    out = np.zeros((S, D), np.float64)
    for r in res.results:
        out += r["out"].astype(np.float64)
    return out.astype(np.float32).reshape(1, S, D)
